# revision 14
# baseline (speedup 1.0000x reference)
"""Dot-product attention pooling kernel for Trainium2 (Bass/Tile).

reference:
    query   = x[:, -1, :]                       # [B, D]
    scores  = einsum('btd,bd->bt', x, query)    # [B, T]
    weights = softmax(scores, axis=1)           # [B, T]
    context = einsum('bt,btd->bd', weights, x)  # [B, D]
    returns (context, weights[:, :, None])

Sharding: pure data parallel over batch across 8 cores (256 samples/core).

Per-core dataflow (per sample, SBUF layout A[p, r, d] with t = 4p + r,
A cast to bf16 during the load DMA):
  1. PE transposes A blocks -> X^T in PSUM (bf16), ACT/DVE copy -> bf16 SBUF.
  2. Scores via 8 stationary matmuls (X^T block stationary, q column moving,
     N=1) -> f32 PSUM scores in [p=t//4, (sample, r=t%4)] layout.
  3. Softmax over t = (partitions x r): DVE reduce over r, GPSIMD
     partition_all_reduce over p, ACT exp, DVE normalize (all f32).
  4. Context: 4 bf16 matmuls per sample (w column stationary, A moving,
     N=256) -> f32 PSUM rows at partition 0, free-sliced, batched DMA out.
Weights leave t-permuted (t = 4p + r ordering); host unpermutes.
"""

import os

import numpy as np

import concourse.bass as bass
import concourse.tile as tile
from concourse import mybir
from concourse.bass_utils import run_bass_kernel_spmd
from concourse.masks import make_identity

B, T, D = 2048, 512, 256
NCORES = 8
BC = B // NCORES  # samples per core
P = 128
R = 4             # t = 4p + r
G = 16            # samples per softmax group
NG = BC // G      # groups per core

F32 = mybir.dt.float32
BF16 = mybir.dt.bfloat16


def split_multi_waits(nc):
    """The nix walrus encodes at most one sync wait per instruction; hoist
    extra waits onto same-engine NoOps inserted just before the instruction."""
    n_split = 0
    for f in nc.m.functions:
        for blk in f.blocks:
            out = []
            for inst in blk.instructions:
                si = getattr(inst, "sync_info", None)
                if si is not None and si.on_wait and len(si.on_wait) > 1:
                    waits = list(si.on_wait)
                    for w in waits[:-1]:
                        nop = mybir.InstNoOp(
                            name=nc.get_next_instruction_name(),
                            ins=[],
                            outs=[],
                            sync_info=mybir.SyncInfo(on_wait=[w], on_update=[]),
                            engine=inst.engine,
                        )
                        nc.register_instruction(nop, overwrite=True)
                        out.append(nop)
                        n_split += 1
                    inst.sync_info = mybir.SyncInfo(
                        on_wait=[waits[-1]], on_update=list(si.on_update)
                    )
                out.append(inst)
            blk.instructions = out
    return n_split


def build_bass():
    nc = bass.Bass(trn_type="TRN2")
    x = nc.dram_tensor("x", [BC, T, D], F32, kind="ExternalInput")
    ctx_out = nc.dram_tensor("ctx", [BC, D], F32, kind="ExternalOutput")
    # weights, t-permuted: wp[p, g, i*R + j] = w[g*G + i, 4p + j]
    wp_out = nc.dram_tensor("wp", [P, NG, G * R], F32, kind="ExternalOutput")

    with tile.TileContext(nc) as tc:
        with (
            tc.tile_pool(name="const", bufs=1) as constp,
            tc.tile_pool(name="apool", bufs=16) as apool,
            tc.tile_pool(name="xtpool", bufs=4) as xtpool,
            tc.tile_pool(name="smpool", bufs=3) as smpool,
            tc.tile_pool(name="csbp", bufs=2) as csbp,
            tc.tile_pool(name="xtps0", bufs=2, space="PSUM") as xtpsp0,
            tc.tile_pool(name="xtps1", bufs=2, space="PSUM") as xtpsp1,
            tc.tile_pool(name="sps", bufs=1, space="PSUM") as spsp,
            tc.tile_pool(name="cps", bufs=2, space="PSUM") as cpsp,
            tc.tile_pool(name="smps", bufs=1, space="PSUM") as smpsp,
        ):
            ident = constp.tile([P, P], BF16)
            make_identity(nc, ident)
            ident_f = constp.tile([P, P], F32)
            make_identity(nc, ident_f)
            ones_row = constp.tile([1, P], F32)
            nc.vector.memset(ones_row, 1.0)
            ones_col = constp.tile([P, 1], F32)
            nc.vector.memset(ones_col, 1.0)

            for g in range(NG):
                b0 = g * G
                # ---- load A tiles: 2 samples per DMA, cast f32 -> bf16 ----
                apairs = []
                for jj in range(G // 2):
                    a_t = apool.tile([P, 2, R, D], BF16, tag="apair")
                    src = x[b0 + 2 * jj : b0 + 2 * jj + 2].rearrange(
                        "b (p r) d -> p b r d", r=R
                    )
                    nc.gpsimd.dma_start(out=a_t, in_=src)
                    apairs.append(a_t)

                # ---- scores for the group ----
                s_grp = spsp.tile([P, G * R], F32, tag="sgrp")
                for i in range(G):
                    a_i = apairs[i // 2][:, i % 2]  # [P, R, D] bf16
                    xt_sb = []
                    for k in range(2):  # d chunk
                        pool_k = xtpsp0 if k == 0 else xtpsp1
                        xt_ps = pool_k.tile([P, 512], BF16, tag=f"xtps{k}")
                        for r in range(R):
                            # A block [p, d] -> XT block [d, u], u = r*128 + p
                            nc.tensor.transpose(
                                xt_ps[:, r * 128 : (r + 1) * 128],
                                a_i[:, r, k * 128 : (k + 1) * 128],
                                ident,
                            )
                        sb = xtpool.tile([P, 512], BF16, tag=f"xtsb{k}")
                        nc.vector.tensor_copy(sb, xt_ps)
                        xt_sb.append(sb)
                    # q = column u=511 (t=511). 8 matmuls, X^T block stationary.
                    for j in range(R):
                        c = i * R + j
                        for k in range(2):
                            nc.tensor.matmul(
                                s_grp[:, c : c + 1],
                                lhsT=xt_sb[k][:, j * 128 : (j + 1) * 128],
                                rhs=xt_sb[k][:, 511:512],
                                start=(k == 0),
                                stop=(k == 1),
                            )

                # ---- softmax over t = (p, j) per sample i ----
                # partition reductions via PE (transpose / ones-matmul),
                # per-sample scalars re-broadcast across partitions via
                # ones-matmul.
                s_v = s_grp.rearrange("p (i j) -> p i j", j=R)
                m1 = smpool.tile([P, G], F32, tag="m1")
                nc.vector.tensor_reduce(
                    m1, s_v, axis=mybir.AxisListType.X, op=mybir.AluOpType.max
                )
                m1t = smpsp.tile([G, P], F32, tag="smps")
                nc.tensor.transpose(m1t, m1, ident_f)
                mxs = smpool.tile([G, 1], F32, tag="mxs")
                nc.vector.tensor_reduce(
                    mxs, m1t, axis=mybir.AxisListType.X, op=mybir.AluOpType.max
                )
                mrow_ps = smpsp.tile([1, G], F32, tag="smps")
                nc.tensor.transpose(mrow_ps, mxs, ident_f[:G, :G])
                mrow = smpool.tile([1, G], F32, tag="mrow")
                nc.scalar.copy(mrow, mrow_ps)
                mb = smpsp.tile([P, G], F32, tag="smps")
                nc.tensor.matmul(mb, lhsT=ones_row, rhs=mrow)
                mbs = smpool.tile([P, G], F32, tag="mbs")
                nc.scalar.copy(mbs, mb)
                ssub = smpool.tile([P, G, R], F32, tag="ssub")
                nc.vector.tensor_tensor(
                    ssub,
                    s_v,
                    mbs[:, :, None].to_broadcast((P, G, R)),
                    mybir.AluOpType.subtract,
                )
                w_sb = smpool.tile([P, G, R], F32, tag="wsb")
                nc.scalar.activation(w_sb, ssub, mybir.ActivationFunctionType.Exp)
                z1 = smpool.tile([P, G], F32, tag="z1")
                nc.vector.tensor_reduce(
                    z1, w_sb, axis=mybir.AxisListType.X, op=mybir.AluOpType.add
                )
                zrow_ps = smpsp.tile([1, G], F32, tag="smps")
                nc.tensor.matmul(zrow_ps, lhsT=ones_col, rhs=z1)
                rz_row = smpool.tile([1, G], F32, tag="rzrow")
                nc.vector.reciprocal(rz_row, zrow_ps)
                rzb = smpsp.tile([P, G], F32, tag="smps")
                nc.tensor.matmul(rzb, lhsT=ones_row, rhs=rz_row)
                nc.vector.tensor_tensor(
                    w_sb,
                    w_sb,
                    rzb[:, :, None].to_broadcast((P, G, R)),
                    mybir.AluOpType.mult,
                )
                w_flat = w_sb.rearrange("p i j -> p (i j)")
                nc.sync.dma_start(out=wp_out[:, g, :], in_=w_flat)
                # bf16 copy of the weights for the context matmul lhsT
                w_bf = smpool.tile([P, G * R], BF16, tag="wbf")
                nc.vector.tensor_copy(w_bf, w_flat)

                # ---- dominant-term precision correction ----
                # the bf16 context matmul rounds x; restore f32 precision of
                # the (overwhelmingly dominant) t=T-1 term:
                #   corr_i = w_i[T-1] * (x_511_f32 - bf16(x_511))
                q32 = smpool.tile([G, D], F32, tag="q32")
                nc.sync.dma_start(out=q32, in_=x[b0 : b0 + G, T - 1, :])
                qbf = smpool.tile([G, D], BF16, tag="qbf")
                nc.vector.tensor_copy(qbf, q32)
                qbf32 = smpool.tile([G, D], F32, tag="qbf32")
                nc.vector.tensor_copy(qbf32, qbf)
                qrf = smpool.tile([G, D], F32, tag="qrf")
                nc.vector.tensor_tensor(
                    qrf, q32, qbf32, mybir.AluOpType.subtract
                )
                # w511 for each sample lives at partition 127, j=3; gather it
                # to [G, 1] via (partition-flexible) DMA.
                w511 = smpool.tile([G, 1], F32, tag="w511")
                nc.gpsimd.dma_start(out=w511, in_=w_sb[127:128, :, 3])
                corr = smpool.tile([G, D], F32, tag="corr")
                nc.vector.tensor_scalar_mul(corr, qrf, w511)

                # ---- context ----
                c_sb = csbp.tile([1, G, D], F32, tag="csb")
                for i in range(G):
                    a_i = apairs[i // 2][:, i % 2]
                    if i % 2 == 0:
                        c_ps = cpsp.tile([1, 2, D], F32, tag="cps")
                    for j in range(R):
                        c = i * R + j
                        nc.tensor.matmul(
                            c_ps[0:1, i % 2],
                            lhsT=w_bf[:, c : c + 1],
                            rhs=a_i[:, j, :],
                            start=(j == 0),
                            stop=(j == R - 1),
                        )
                    if i % 2 == 1:
                        pair = i // 2
                        dst = c_sb[0:1, 2 * pair : 2 * pair + 2, :]
                        nc.scalar.copy(dst, c_ps)
                # accumulate the precision correction into c_sb (partition
                # scatter via DMA), then write out
                nc.gpsimd.dma_start(
                    out=c_sb, in_=corr, accum_op=mybir.AluOpType.add
                )
                nc.sync.dma_start(out=ctx_out[b0 : b0 + G, :], in_=c_sb)

    split_multi_waits(nc)
    return nc


_NC_CACHE = {}


def _get_nc():
    if "nc" not in _NC_CACHE:
        _NC_CACHE["nc"] = build_bass()
    return _NC_CACHE["nc"]


def kernel(lstm_outputs: np.ndarray):
    x = np.ascontiguousarray(np.asarray(lstm_outputs, dtype=np.float32))
    assert x.shape == (B, T, D), x.shape

    nc = _get_nc()
    in_maps = [
        {"x": x[c * BC : (c + 1) * BC]} for c in range(NCORES)
    ]
    trace = bool(int(os.environ.get("BASS_KERNEL_TRACE", "0")))
    res = run_bass_kernel_spmd(
        nc, in_maps, core_ids=list(range(NCORES)), trace=trace
    )
    kernel.last_result = res

    ctx = np.concatenate([r["ctx"] for r in res.results], axis=0)  # [B, D]
    # unpermute weights: wp [P, NG, G*R] -> w [BC, T] per core
    ws = []
    for r in res.results:
        wp = r["wp"].reshape(P, NG, G, R)  # [p, g, i, j]
        w = wp.transpose(1, 2, 0, 3).reshape(BC, T)  # [b, t=4p+j]
        ws.append(w)
    weights = np.concatenate(ws, axis=0)[:, :, None]  # [B, T, 1]
    return ctx.astype(np.float32), weights.astype(np.float32)


kernel.last_result = None
